# revision 27
# baseline (speedup 1.0000x reference)
"""Trainium2 Bass kernel for the 8x8-block rfft2 magnitude ("DCT") layer.

Computes, for input x [32,1,512,512] f32 and freq_weights [64] f32:
  per 8x8 spatial block: |rfft2(block, norm='ortho')| -> 40 freq bins,
  scaled by sigmoid(freq_weights)[:40], zero-padded to 64 channels.
Output: [32, 64, 64, 64] f32 (channels 40..63 are zero).

Strategy (pure data parallel, 4 images per core on 8 cores):
  The per-block 2D DFT is separable.  Per 128-row x 512-col slab:
    stage 1 (one matmul per 128-col chunk): data is the *stationary*
      operand, a block-diagonal cos/sin matrix streams:
      Z = A_chunk.T @ W1 -> vertical DFT of every row-block with the
      output transposed so j (intra-block col) is on partitions.
    stage 2 (two accumulating matmuls per chunk): Z_re/Z_im stationary,
      [C2|S2|0] / [-S2|C2|0] streaming -> Re/Im of the 2D DFT laid out
      [(bi,u), (v,bj)], matching output memory order after (u,v) merge.
  Matmul operands use float32r (TF32-class, ~1e-4 rel err, 4x rate at
  N>=256).  Magnitude on DVE (squares/add) + ACT (sqrt only, no table
  swap), sigmoid-weighting on DVE, one store DMA per slab with 256B
  runs.  Channels 40..63 rely on the runtime pre-zeroing outputs.
"""

import math
import numpy as np
from contextlib import ExitStack

import concourse.bass as bass
import concourse.bacc as bacc
import concourse.mybir as mybir
from concourse import tile
from concourse.bass_utils import run_bass_kernel_spmd

F32 = mybir.dt.float32
F32R = mybir.dt.float32r

N_CORES = 8
IMGS_PER_CORE = 4  # 32 / 8
SLABS_PER_IMG = 4  # 512 rows / 128


def _build_host_matrices(freq_weights: np.ndarray):
    """Block-diagonal DFT coefficient matrices + sigmoid weight tile."""
    p = np.arange(128)
    # W1 [128, 256]: row p=(bi,i); col n=(reim, bi2, u). Vertical DFT, /8.
    bi_p, i_p = p // 8, p % 8
    n = np.arange(256)
    reim_n, r = n // 128, n % 128
    bi2_n, u_n = r // 8, r % 8
    ang1 = 2.0 * math.pi * np.outer(i_p, u_n) / 8.0
    W1 = np.where(reim_n[None, :] == 0, np.cos(ang1), np.sin(ang1)) / 8.0
    W1 *= (bi_p[:, None] == bi2_n[None, :])
    W1 = W1.astype(np.float32)

    # C2/S2 [128, 80]: row p=(bj,j); col m=(v, bj2). Horizontal DFT.
    bj_p, j_p = p // 8, p % 8
    m = np.arange(80)
    v_m, bj2_m = m // 16, m % 16
    ang2 = 2.0 * math.pi * np.outer(j_p, v_m) / 8.0
    blk = (bj_p[:, None] == bj2_m[None, :])
    C2 = (np.cos(ang2) * blk).astype(np.float32)
    S2 = (np.sin(ang2) * blk).astype(np.float32)
    z96 = np.zeros((128, 96), dtype=np.float32)
    # padded to N=256 so float32r streams at 1 cycle/row
    CS2P = np.concatenate([C2, S2, z96], axis=1)
    SNC2P = np.concatenate([-S2, C2, z96], axis=1)

    # Wtile [128, 320]: p=(bi,u), f=(v,bj) -> sigmoid(freq_weights)[u*5+v]
    w = 1.0 / (1.0 + np.exp(-freq_weights.astype(np.float64)))
    u_idx = np.arange(128) % 8
    v_idx = np.arange(320) // 64
    Wtile = w[u_idx[:, None] * 5 + v_idx[None, :]].astype(np.float32)
    return W1, CS2P, SNC2P, Wtile


_NC_CACHE = None


def _build_bass(n_imgs: int = IMGS_PER_CORE, repeat: int = 1, cfg: dict = None):
    cfg = dict(cfg or {})
    n_dve_cop = cfg.get("dve_cop", 4)   # chunks 0..n-1 copy on DVE, rest ACT
    n_dve_sq = cfg.get("dve_sq", 0)     # chunks 0..n-1 square on DVE, rest ACT
    add_eng = cfg.get("add", "dve")
    wm_eng = cfg.get("wm", "pool")
    psz_b = cfg.get("psz", 2)
    pso_b = cfg.get("pso", 6)
    ab = cfg.get("a", 8)
    zb = cfg.get("z", 12)
    sqb = cfg.get("sq", 8)
    magb = cfg.get("mag", 8)
    nc = bacc.Bacc("TRN2", target_bir_lowering=False)
    x = nc.dram_tensor("x", [n_imgs * 512, 512], F32R, kind="ExternalInput")
    cst = nc.dram_tensor("cst", [128, 1088], F32R, kind="ExternalInput")
    out = nc.dram_tensor(
        "out", [n_imgs, 64, 64, 64], F32, kind="ExternalOutput"
    )

    # store view: [img, bi_l, s, u, v, bj]; (u,v) merges into one AP dim
    out40 = out[:, 0:40, :, :].rearrange(
        "b (u v) (s p) q -> b p s u v q", u=8, v=5, s=SLABS_PER_IMG, p=16
    )

    with tile.TileContext(nc) as tc, ExitStack() as ctx:
        consts = ctx.enter_context(tc.tile_pool(name="consts", bufs=1))
        a_pool = ctx.enter_context(tc.tile_pool(name="a", bufs=ab))
        z_pool = ctx.enter_context(tc.tile_pool(name="z", bufs=zb))
        sq_pool = ctx.enter_context(tc.tile_pool(name="sq", bufs=sqb))
        mag_pool = ctx.enter_context(tc.tile_pool(name="mag", bufs=magb))
        psz_pool = ctx.enter_context(tc.tile_pool(name="psz", bufs=psz_b, space="PSUM"))
        pso_pool = ctx.enter_context(tc.tile_pool(name="pso", bufs=pso_b, space="PSUM"))

        cst_t = consts.tile([128, 1088], F32R, tag="cst")
        nc.sync.dma_start(cst_t[:], cst[:])
        w1_t = cst_t[:, 0:256]
        cs2_t = cst_t[:, 256:512]
        snc2_t = cst_t[:, 512:768]
        wt_t = cst_t[:, 768:1088]

        def emit_head(img, s):
            a_t = a_pool.tile([128, 512], F32R)
            row0 = img * 512 + s * 128
            nc.scalar.dma_start(a_t[:], x[row0 : row0 + 128, :])
            sq = sq_pool.tile([128, 640], F32, tag="sq")
            # stage 1 for all chunks first: PE never stalls on the
            # DVE copy of the same chunk's Z
            zts = []
            for c in range(4):
                psz = psz_pool.tile([128, 256], F32, tag="psz")
                nc.tensor.matmul(
                    psz[:],
                    a_t[:, 128 * c : 128 * (c + 1)],
                    w1_t,
                    start=True,
                    stop=True,
                )
                z_t = z_pool.tile([128, 256], F32R)
                if c < n_dve_cop:
                    nc.vector.tensor_copy(z_t[:], psz[:])
                else:
                    nc.scalar.copy(z_t[:], psz[:])
                zts.append(z_t)
            for c in range(4):
                z_t = zts[c]
                o2 = pso_pool.tile([128, 256], F32, tag="o2")
                nc.tensor.matmul(o2[:], z_t[:, 0:128], cs2_t, start=True, stop=False)
                nc.tensor.matmul(o2[:], z_t[:, 128:256], snc2_t, start=False, stop=True)
                # squares split DVE/ACT into the slab-level sq tile
                if c < n_dve_sq:
                    nc.vector.tensor_scalar(
                        sq[:, 160 * c : 160 * (c + 1)], o2[:, 0:160],
                        2.0, None, mybir.AluOpType.pow,
                    )
                else:
                    nc.scalar.square(sq[:, 160 * c : 160 * (c + 1)], o2[:, 0:160])
            return sq

        def emit_tail(img, s, sq):
            # one add / sqrt / weight-mul per slab (batched over chunks)
            root = mag_pool.tile([128, 320], F32, tag="root")
            ssum = sq_pool.tile([128, 320], F32, tag="ssum")
            sqv = sq[:].rearrange("p (c h g) -> p c h g", c=4, h=2, g=80)
            add_fn = nc.gpsimd.tensor_add if add_eng == "pool" else nc.vector.tensor_add
            add_fn(
                ssum[:].rearrange("p (c g) -> p c g", c=4, g=80),
                sqv[:, :, 0],
                sqv[:, :, 1],
            )
            # write v-major into root: free = v*64 + 16*c + (0..16)
            nc.scalar.sqrt(
                root[:].rearrange("p (v c q) -> p c v q", v=5, c=4, q=16),
                ssum[:].rearrange("p (c v q) -> p c v q", c=4, v=5, q=16),
            )
            magf = mag_pool.tile([128, 320], F32, tag="magf")
            (nc.gpsimd.tensor_mul if wm_eng == "pool" else nc.vector.tensor_mul)(magf[:], root[:], wt_t)
            nc.sync.dma_start(out40[img, :, s], magf[:])

        rep_ctx = tc.For_i(0, repeat, 1) if repeat > 1 else None
        if rep_ctx is not None:
            rep_ctx.__enter__()
        # software-pipelined emission: tail of slab k emitted after head k+1
        slabs = [(img, s) for img in range(n_imgs) for s in range(SLABS_PER_IMG)]
        pend = []
        for img, s in slabs:
            sq = emit_head(img, s)
            pend.append((img, s, sq))
            if len(pend) > 1:
                emit_tail(*pend.pop(0))
        while pend:
            emit_tail(*pend.pop(0))
        if rep_ctx is not None:
            rep_ctx.__exit__(None, None, None)
    nc.finalize()
    return nc


def kernel(x: np.ndarray, freq_weights: np.ndarray) -> np.ndarray:
    global _NC_CACHE
    x = np.ascontiguousarray(np.asarray(x, dtype=np.float32))
    freq_weights = np.asarray(freq_weights, dtype=np.float32)
    B = x.shape[0]
    assert x.shape == (32, 1, 512, 512) and freq_weights.shape == (64,)

    W1, CS2P, SNC2P, Wtile = _build_host_matrices(freq_weights)
    cst = np.concatenate([W1, CS2P, SNC2P, Wtile], axis=1)
    if _NC_CACHE is None:
        _NC_CACHE = _build_bass()
    nc = _NC_CACHE

    per = B // N_CORES
    in_maps = []
    for k in range(N_CORES):
        in_maps.append(
            {
                "x": x[k * per : (k + 1) * per].reshape(per * 512, 512),
                "cst": cst,
            }
        )
    res = run_bass_kernel_spmd(nc, in_maps, list(range(N_CORES))).results
    out = np.concatenate([res[k]["out"] for k in range(N_CORES)], axis=0)
    return out.astype(np.float32)


# revision 28
# speedup vs baseline: 1.0580x; 1.0580x over previous
"""Trainium2 Bass kernel for the 8x8-block rfft2 magnitude ("DCT") layer.

Computes, for input x [32,1,512,512] f32 and freq_weights [64] f32:
  per 8x8 spatial block: |rfft2(block, norm='ortho')| -> 40 freq bins,
  scaled by sigmoid(freq_weights)[:40], zero-padded to 64 channels.
Output: [32, 64, 64, 64] f32 (channels 40..63 are zero).

Strategy (pure data parallel, 4 images per core on 8 cores):
  The per-block 2D DFT is separable.  Per 128-row x 512-col slab:
    stage 1 (one matmul per 128-col chunk): data is the *stationary*
      operand, a block-diagonal cos/sin matrix streams:
      Z = A_chunk.T @ W1 -> vertical DFT of every row-block with the
      output transposed so j (intra-block col) is on partitions.
    stage 2 (two accumulating matmuls per chunk): Z_re/Z_im stationary,
      [C2|S2|0] / [-S2|C2|0] streaming -> Re/Im of the 2D DFT laid out
      [(bi,u), (v,bj)], matching output memory order after (u,v) merge.
  Matmul operands use float32r (TF32-class, ~1e-4 rel err, 4x rate at
  N>=256).  Magnitude on DVE (squares/add) + ACT (sqrt only, no table
  swap), sigmoid-weighting on DVE, one store DMA per slab with 256B
  runs.  Channels 40..63 rely on the runtime pre-zeroing outputs.
"""

import math
import numpy as np
from contextlib import ExitStack

import concourse.bass as bass
import concourse.bacc as bacc
import concourse.mybir as mybir
from concourse import tile
from concourse.bass_utils import run_bass_kernel_spmd

F32 = mybir.dt.float32
F32R = mybir.dt.float32r

N_CORES = 8
IMGS_PER_CORE = 4  # 32 / 8
SLABS_PER_IMG = 4  # 512 rows / 128


def _build_host_matrices(freq_weights: np.ndarray):
    """Block-diagonal DFT coefficient matrices + sigmoid weight tile."""
    p = np.arange(128)
    # W1 [128, 256]: row p=(bi,i); col n=(reim, bi2, u). Vertical DFT, /8.
    bi_p, i_p = p // 8, p % 8
    n = np.arange(256)
    reim_n, r = n // 128, n % 128
    bi2_n, u_n = r // 8, r % 8
    ang1 = 2.0 * math.pi * np.outer(i_p, u_n) / 8.0
    W1 = np.where(reim_n[None, :] == 0, np.cos(ang1), np.sin(ang1)) / 8.0
    W1 *= (bi_p[:, None] == bi2_n[None, :])
    W1 = W1.astype(np.float32)

    # C2/S2 [128, 80]: row p=(bj,j); col m=(v, bj2). Horizontal DFT.
    bj_p, j_p = p // 8, p % 8
    m = np.arange(80)
    v_m, bj2_m = m // 16, m % 16
    ang2 = 2.0 * math.pi * np.outer(j_p, v_m) / 8.0
    blk = (bj_p[:, None] == bj2_m[None, :])
    C2 = (np.cos(ang2) * blk).astype(np.float32)
    S2 = (np.sin(ang2) * blk).astype(np.float32)
    z96 = np.zeros((128, 96), dtype=np.float32)
    # padded to N=256 so float32r streams at 1 cycle/row
    CS2P = np.concatenate([C2, S2, z96], axis=1)
    SNC2P = np.concatenate([-S2, C2, z96], axis=1)

    # Wtile [128, 320]: p=(bi,u), f=(v,bj) -> sigmoid(freq_weights)[u*5+v]
    w = 1.0 / (1.0 + np.exp(-freq_weights.astype(np.float64)))
    u_idx = np.arange(128) % 8
    v_idx = np.arange(320) // 64
    Wtile = w[u_idx[:, None] * 5 + v_idx[None, :]].astype(np.float32)
    return W1, CS2P, SNC2P, Wtile


_NC_CACHE = None


def _build_bass(n_imgs: int = IMGS_PER_CORE, repeat: int = 1, cfg: dict = None):
    cfg = dict(cfg or {})
    n_dve_cop = cfg.get("dve_cop", 4)   # chunks 0..n-1 copy on DVE, rest ACT
    n_dve_sq = cfg.get("dve_sq", 0)     # chunks 0..n-1 square on DVE, rest ACT
    add_eng = cfg.get("add", "dve")
    wm_eng = cfg.get("wm", "pool")
    psz_b = cfg.get("psz", 2)
    pso_b = cfg.get("pso", 6)
    ab = cfg.get("a", 8)
    zb = cfg.get("z", 12)
    sqb = cfg.get("sq", 8)
    magb = cfg.get("mag", 8)
    nc = bacc.Bacc("TRN2", target_bir_lowering=False)
    x = nc.dram_tensor("x", [n_imgs * 512, 512], F32R, kind="ExternalInput")
    cst = nc.dram_tensor("cst", [128, 1088], F32R, kind="ExternalInput")
    out = nc.dram_tensor(
        "out", [n_imgs, 64, 64, 64], F32, kind="ExternalOutput"
    )

    # store view: [img, bi_l, s, u, v, bj]; (u,v) merges into one AP dim
    out40 = out[:, 0:40, :, :].rearrange(
        "b (u v) (s p) q -> b p s u v q", u=8, v=5, s=SLABS_PER_IMG, p=16
    )

    with tile.TileContext(nc) as tc, ExitStack() as ctx:
        consts = ctx.enter_context(tc.tile_pool(name="consts", bufs=1))
        a_pool = ctx.enter_context(tc.tile_pool(name="a", bufs=ab))
        z_pool = ctx.enter_context(tc.tile_pool(name="z", bufs=zb))
        sq_pool = ctx.enter_context(tc.tile_pool(name="sq", bufs=sqb))
        mag_pool = ctx.enter_context(tc.tile_pool(name="mag", bufs=magb))
        psz_pool = ctx.enter_context(tc.tile_pool(name="psz", bufs=psz_b, space="PSUM"))
        pso_pool = ctx.enter_context(tc.tile_pool(name="pso", bufs=pso_b, space="PSUM"))

        cst_t = consts.tile([128, 1088], F32R, tag="cst")
        nc.sync.dma_start(cst_t[:], cst[:])
        w1_t = cst_t[:, 0:256]
        cs2_t = cst_t[:, 256:512]
        snc2_t = cst_t[:, 512:768]
        wt_t = cst_t[:, 768:1088]

        def emit_head(img, s):
            a_t = a_pool.tile([128, 512], F32R)
            row0 = img * 512 + s * 128
            nc.scalar.dma_start(a_t[:], x[row0 : row0 + 128, :])
            sq = sq_pool.tile([128, 640], F32, tag="sq")
            # stage 1 for all chunks first: PE never stalls on the
            # DVE copy of the same chunk's Z
            zts = []
            for c in range(4):
                psz = psz_pool.tile([128, 256], F32, tag="psz")
                nc.tensor.matmul(
                    psz[:],
                    a_t[:, 128 * c : 128 * (c + 1)],
                    w1_t,
                    start=True,
                    stop=True,
                )
                z_t = z_pool.tile([128, 256], F32R)
                if c < n_dve_cop:
                    nc.vector.tensor_copy(z_t[:], psz[:])
                else:
                    nc.scalar.copy(z_t[:], psz[:])
                zts.append(z_t)
            for c in range(4):
                z_t = zts[c]
                o2 = pso_pool.tile([128, 256], F32, tag="o2")
                nc.tensor.matmul(o2[:], z_t[:, 0:128], cs2_t, start=True, stop=False)
                nc.tensor.matmul(o2[:], z_t[:, 128:256], snc2_t, start=False, stop=True)
                # squares split DVE/ACT into the slab-level sq tile
                if c < n_dve_sq:
                    nc.vector.tensor_scalar(
                        sq[:, 160 * c : 160 * (c + 1)], o2[:, 0:160],
                        2.0, None, mybir.AluOpType.pow,
                    )
                else:
                    nc.scalar.square(sq[:, 160 * c : 160 * (c + 1)], o2[:, 0:160])
            return sq

        def emit_tail(img, s, sq):
            # one add / sqrt / weight-mul per slab (batched over chunks)
            root = mag_pool.tile([128, 320], F32, tag="root")
            ssum = sq_pool.tile([128, 320], F32, tag="ssum")
            sqv = sq[:].rearrange("p (c h g) -> p c h g", c=4, h=2, g=80)
            add_fn = nc.gpsimd.tensor_add if add_eng == "pool" else nc.vector.tensor_add
            add_fn(
                ssum[:].rearrange("p (c g) -> p c g", c=4, g=80),
                sqv[:, :, 0],
                sqv[:, :, 1],
            )
            # write v-major into root: free = v*64 + 16*c + (0..16)
            nc.scalar.sqrt(
                root[:].rearrange("p (v c q) -> p c v q", v=5, c=4, q=16),
                ssum[:].rearrange("p (c v q) -> p c v q", c=4, v=5, q=16),
            )
            magf = mag_pool.tile([128, 320], F32, tag="magf")
            (nc.gpsimd.tensor_mul if wm_eng == "pool" else nc.vector.tensor_mul)(magf[:], root[:], wt_t)
            nc.sync.dma_start(out40[img, :, s], magf[:])

        rep_ctx = tc.For_i(0, repeat, 1) if repeat > 1 else None
        if rep_ctx is not None:
            rep_ctx.__enter__()
        # software-pipelined emission: tail of slab k emitted after head k+1
        depth = cfg.get("depth", 1)
        slabs = [(img, s) for img in range(n_imgs) for s in range(SLABS_PER_IMG)]
        pend = []
        for img, s in slabs:
            sq = emit_head(img, s)
            pend.append((img, s, sq))
            if len(pend) > depth:
                emit_tail(*pend.pop(0))
        while pend:
            emit_tail(*pend.pop(0))
        if rep_ctx is not None:
            rep_ctx.__exit__(None, None, None)
    nc.finalize()
    return nc


def kernel(x: np.ndarray, freq_weights: np.ndarray) -> np.ndarray:
    global _NC_CACHE
    x = np.ascontiguousarray(np.asarray(x, dtype=np.float32))
    freq_weights = np.asarray(freq_weights, dtype=np.float32)
    B = x.shape[0]
    assert x.shape == (32, 1, 512, 512) and freq_weights.shape == (64,)

    W1, CS2P, SNC2P, Wtile = _build_host_matrices(freq_weights)
    cst = np.concatenate([W1, CS2P, SNC2P, Wtile], axis=1)
    if _NC_CACHE is None:
        _NC_CACHE = _build_bass()
    nc = _NC_CACHE

    per = B // N_CORES
    in_maps = []
    for k in range(N_CORES):
        in_maps.append(
            {
                "x": x[k * per : (k + 1) * per].reshape(per * 512, 512),
                "cst": cst,
            }
        )
    res = run_bass_kernel_spmd(nc, in_maps, list(range(N_CORES))).results
    out = np.concatenate([res[k]["out"] for k in range(N_CORES)], axis=0)
    return out.astype(np.float32)


# revision 29
# speedup vs baseline: 1.0599x; 1.0017x over previous
"""Trainium2 Bass kernel for the 8x8-block rfft2 magnitude ("DCT") layer.

Computes, for input x [32,1,512,512] f32 and freq_weights [64] f32:
  per 8x8 spatial block: |rfft2(block, norm='ortho')| -> 40 freq bins,
  scaled by sigmoid(freq_weights)[:40], zero-padded to 64 channels.
Output: [32, 64, 64, 64] f32 (channels 40..63 are zero).

Strategy (pure data parallel, 4 images per core on 8 cores):
  The per-block 2D DFT is separable.  Per 128-row x 512-col slab:
    stage 1 (one matmul per 128-col chunk): data is the *stationary*
      operand, a block-diagonal cos/sin matrix streams:
      Z = A_chunk.T @ W1 -> vertical DFT of every row-block with the
      output transposed so j (intra-block col) is on partitions.
    stage 2 (two accumulating matmuls per chunk): Z_re/Z_im stationary,
      [C2|S2|0] / [-S2|C2|0] streaming -> Re/Im of the 2D DFT laid out
      [(bi,u), (v,bj)], matching output memory order after (u,v) merge.
  Matmul operands use float32r (TF32-class, ~2e-4 rel err, 4x rate at
  N>=256).  PSUM->SBUF Z copies on DVE, squares+sqrt on ACT, re/im add
  on DVE, sigmoid-weighting on GPSIMD; input loads on the ACT HWDGE
  ring, one store DMA per slab (256B runs) on the SP HWDGE ring so
  stores never head-of-line block prefetch loads.  Channels 40..63
  rely on the runtime pre-zeroing ExternalOutput buffers.
  Measured ~55us/core steady-state (8 cores, 4 images each) vs ~18-27us
  memory roofline; engines balanced: DVE ~31us, ACT ~30us, DMA ~28us.
"""

import math
import numpy as np
from contextlib import ExitStack

import concourse.bacc as bacc
import concourse.mybir as mybir
from concourse import tile
from concourse.bass_utils import run_bass_kernel_spmd

F32 = mybir.dt.float32
F32R = mybir.dt.float32r

N_CORES = 8
IMGS_PER_CORE = 4  # 32 / 8
SLABS_PER_IMG = 4  # 512 rows / 128


def _build_host_matrices(freq_weights: np.ndarray):
    """Block-diagonal DFT coefficient matrices + sigmoid weight tile."""
    p = np.arange(128)
    # W1 [128, 256]: row p=(bi,i); col n=(reim, bi2, u). Vertical DFT, /8.
    bi_p, i_p = p // 8, p % 8
    n = np.arange(256)
    reim_n, r = n // 128, n % 128
    bi2_n, u_n = r // 8, r % 8
    ang1 = 2.0 * math.pi * np.outer(i_p, u_n) / 8.0
    W1 = np.where(reim_n[None, :] == 0, np.cos(ang1), np.sin(ang1)) / 8.0
    W1 *= (bi_p[:, None] == bi2_n[None, :])
    W1 = W1.astype(np.float32)

    # C2/S2 [128, 80]: row p=(bj,j); col m=(v, bj2). Horizontal DFT.
    bj_p, j_p = p // 8, p % 8
    m = np.arange(80)
    v_m, bj2_m = m // 16, m % 16
    ang2 = 2.0 * math.pi * np.outer(j_p, v_m) / 8.0
    blk = (bj_p[:, None] == bj2_m[None, :])
    C2 = (np.cos(ang2) * blk).astype(np.float32)
    S2 = (np.sin(ang2) * blk).astype(np.float32)
    z96 = np.zeros((128, 96), dtype=np.float32)
    # padded to N=256 so float32r streams at 1 cycle/row
    CS2P = np.concatenate([C2, S2, z96], axis=1)
    SNC2P = np.concatenate([-S2, C2, z96], axis=1)

    # Wtile [128, 320]: p=(bi,u), f=(v,bj) -> sigmoid(freq_weights)[u*5+v]
    w = 1.0 / (1.0 + np.exp(-freq_weights.astype(np.float64)))
    u_idx = np.arange(128) % 8
    v_idx = np.arange(320) // 64
    Wtile = w[u_idx[:, None] * 5 + v_idx[None, :]].astype(np.float32)
    return W1, CS2P, SNC2P, Wtile


_NC_CACHE = None


def _build_bass(n_imgs: int = IMGS_PER_CORE, repeat: int = 1, cfg: dict = None):
    cfg = dict(cfg or {})
    n_dve_cop = cfg.get("dve_cop", 4)   # chunks 0..n-1 copy on DVE, rest ACT
    n_dve_sq = cfg.get("dve_sq", 0)     # chunks 0..n-1 square on DVE, rest ACT
    add_eng = cfg.get("add", "dve")
    wm_eng = cfg.get("wm", "pool")
    psz_b = cfg.get("psz", 3)
    pso_b = cfg.get("pso", 5)
    ab = cfg.get("a", 8)
    zb = cfg.get("z", 12)
    sqb = cfg.get("sq", 8)
    magb = cfg.get("mag", 8)
    nc = bacc.Bacc("TRN2", target_bir_lowering=False)
    x = nc.dram_tensor("x", [n_imgs * 512, 512], F32R, kind="ExternalInput")
    cst = nc.dram_tensor("cst", [128, 1088], F32R, kind="ExternalInput")
    out = nc.dram_tensor(
        "out", [n_imgs, 64, 64, 64], F32, kind="ExternalOutput"
    )

    # store view: [img, bi_l, s, u, v, bj]; (u,v) merges into one AP dim
    out40 = out[:, 0:40, :, :].rearrange(
        "b (u v) (s p) q -> b p s u v q", u=8, v=5, s=SLABS_PER_IMG, p=16
    )

    with tile.TileContext(nc) as tc, ExitStack() as ctx:
        consts = ctx.enter_context(tc.tile_pool(name="consts", bufs=1))
        a_pool = ctx.enter_context(tc.tile_pool(name="a", bufs=ab))
        z_pool = ctx.enter_context(tc.tile_pool(name="z", bufs=zb))
        sq_pool = ctx.enter_context(tc.tile_pool(name="sq", bufs=sqb))
        mag_pool = ctx.enter_context(tc.tile_pool(name="mag", bufs=magb))
        psz_pool = ctx.enter_context(tc.tile_pool(name="psz", bufs=psz_b, space="PSUM"))
        pso_pool = ctx.enter_context(tc.tile_pool(name="pso", bufs=pso_b, space="PSUM"))

        cst_t = consts.tile([128, 1088], F32R, tag="cst")
        nc.sync.dma_start(cst_t[:], cst[:])
        w1_t = cst_t[:, 0:256]
        cs2_t = cst_t[:, 256:512]
        snc2_t = cst_t[:, 512:768]
        wt_t = cst_t[:, 768:1088]

        def emit_head(img, s):
            a_t = a_pool.tile([128, 512], F32R)
            row0 = img * 512 + s * 128
            nc.scalar.dma_start(a_t[:], x[row0 : row0 + 128, :])
            sq = sq_pool.tile([128, 640], F32, tag="sq")
            # stage 1 for all chunks first: PE never stalls on the
            # DVE copy of the same chunk's Z
            zts = []
            for c in range(4):
                psz = psz_pool.tile([128, 256], F32, tag="psz")
                nc.tensor.matmul(
                    psz[:],
                    a_t[:, 128 * c : 128 * (c + 1)],
                    w1_t,
                    start=True,
                    stop=True,
                )
                z_t = z_pool.tile([128, 256], F32R)
                if c < n_dve_cop:
                    nc.vector.tensor_copy(z_t[:], psz[:])
                else:
                    nc.scalar.copy(z_t[:], psz[:])
                zts.append(z_t)
            for c in range(4):
                z_t = zts[c]
                o2 = pso_pool.tile([128, 256], F32, tag="o2")
                nc.tensor.matmul(o2[:], z_t[:, 0:128], cs2_t, start=True, stop=False)
                nc.tensor.matmul(o2[:], z_t[:, 128:256], snc2_t, start=False, stop=True)
                # squares split DVE/ACT into the slab-level sq tile
                if c < n_dve_sq:
                    nc.vector.tensor_scalar(
                        sq[:, 160 * c : 160 * (c + 1)], o2[:, 0:160],
                        2.0, None, mybir.AluOpType.pow,
                    )
                else:
                    nc.scalar.square(sq[:, 160 * c : 160 * (c + 1)], o2[:, 0:160])
            return sq

        def emit_tail(img, s, sq):
            # one add / sqrt / weight-mul per slab (batched over chunks)
            root = mag_pool.tile([128, 320], F32, tag="root")
            ssum = sq_pool.tile([128, 320], F32, tag="ssum")
            sqv = sq[:].rearrange("p (c h g) -> p c h g", c=4, h=2, g=80)
            add_fn = nc.gpsimd.tensor_add if add_eng == "pool" else nc.vector.tensor_add
            add_fn(
                ssum[:].rearrange("p (c g) -> p c g", c=4, g=80),
                sqv[:, :, 0],
                sqv[:, :, 1],
            )
            # write v-major into root: free = v*64 + 16*c + (0..16)
            nc.scalar.sqrt(
                root[:].rearrange("p (v c q) -> p c v q", v=5, c=4, q=16),
                ssum[:].rearrange("p (c v q) -> p c v q", c=4, v=5, q=16),
            )
            magf = mag_pool.tile([128, 320], F32, tag="magf")
            (nc.gpsimd.tensor_mul if wm_eng == "pool" else nc.vector.tensor_mul)(magf[:], root[:], wt_t)
            nc.sync.dma_start(out40[img, :, s], magf[:])

        rep_ctx = tc.For_i(0, repeat, 1) if repeat > 1 else None
        if rep_ctx is not None:
            rep_ctx.__enter__()
        # software-pipelined emission: tail of slab k emitted after head k+1
        depth = cfg.get("depth", 0)
        slabs = [(img, s) for img in range(n_imgs) for s in range(SLABS_PER_IMG)]
        pend = []
        for img, s in slabs:
            sq = emit_head(img, s)
            pend.append((img, s, sq))
            if len(pend) > depth:
                emit_tail(*pend.pop(0))
        while pend:
            emit_tail(*pend.pop(0))
        if rep_ctx is not None:
            rep_ctx.__exit__(None, None, None)
    nc.finalize()
    return nc


def kernel(x: np.ndarray, freq_weights: np.ndarray) -> np.ndarray:
    global _NC_CACHE
    x = np.ascontiguousarray(np.asarray(x, dtype=np.float32))
    freq_weights = np.asarray(freq_weights, dtype=np.float32)
    B = x.shape[0]
    assert x.shape == (32, 1, 512, 512) and freq_weights.shape == (64,)

    W1, CS2P, SNC2P, Wtile = _build_host_matrices(freq_weights)
    cst = np.concatenate([W1, CS2P, SNC2P, Wtile], axis=1)
    if _NC_CACHE is None:
        _NC_CACHE = _build_bass()
    nc = _NC_CACHE

    per = B // N_CORES
    in_maps = []
    for k in range(N_CORES):
        in_maps.append(
            {
                "x": x[k * per : (k + 1) * per].reshape(per * 512, 512),
                "cst": cst,
            }
        )
    res = run_bass_kernel_spmd(nc, in_maps, list(range(N_CORES))).results
    out = np.concatenate([res[k]["out"] for k in range(N_CORES)], axis=0)
    return out.astype(np.float32)


# revision 30
# speedup vs baseline: 1.1278x; 1.0641x over previous
"""Trainium2 Bass kernel for the 8x8-block rfft2 magnitude ("DCT") layer.

Computes, for input x [32,1,512,512] f32 and freq_weights [64] f32:
  per 8x8 spatial block: |rfft2(block, norm='ortho')| -> 40 freq bins,
  scaled by sigmoid(freq_weights)[:40], zero-padded to 64 channels.
Output: [32, 64, 64, 64] f32 (channels 40..63 are zero).

Strategy (pure data parallel, 4 images per core on 8 cores):
  The per-block 2D DFT is separable.  Per 128-row x 512-col slab:
    stage 1 (one matmul per 128-col chunk): data is the *stationary*
      operand, a block-diagonal cos/sin matrix streams:
      Z = A_chunk.T @ W1 -> vertical DFT of every row-block with the
      output transposed so j (intra-block col) is on partitions.
    stage 2 (two accumulating matmuls per chunk): Z_re/Z_im stationary,
      [C2|S2|0] / [-S2|C2|0] streaming -> Re/Im of the 2D DFT laid out
      [(bi,u), (v,bj)], matching output memory order after (u,v) merge.
  Matmul operands use float32r (TF32-class, ~2e-4 rel err, 4x rate at
  N>=256).  PSUM->SBUF Z copies on DVE, squares+sqrt on ACT, re/im add
  on DVE, sigmoid-weighting on GPSIMD; input loads on the ACT HWDGE
  ring, one store DMA per slab (256B runs) on the SP HWDGE ring so
  stores never head-of-line block prefetch loads.  Channels 40..63
  rely on the runtime pre-zeroing ExternalOutput buffers.
  Measured ~55us/core steady-state (8 cores, 4 images each) vs ~18-27us
  memory roofline; engines balanced: DVE ~31us, ACT ~30us, DMA ~28us.
"""

import math
import numpy as np
from contextlib import ExitStack

import concourse.bacc as bacc
import concourse.mybir as mybir
from concourse import tile
from concourse.bass_utils import run_bass_kernel_spmd

F32 = mybir.dt.float32
F32R = mybir.dt.float32r

N_CORES = 8
IMGS_PER_CORE = 4  # 32 / 8
SLABS_PER_IMG = 4  # 512 rows / 128


def _build_host_matrices(freq_weights: np.ndarray):
    """Block-diagonal DFT coefficient matrices + sigmoid weight tile."""
    p = np.arange(128)
    # W1 [128, 256]: row p=(bi,i); col n=(reim, bi2, u). Vertical DFT, /8.
    bi_p, i_p = p // 8, p % 8
    n = np.arange(256)
    reim_n, r = n // 128, n % 128
    bi2_n, u_n = r // 8, r % 8
    ang1 = 2.0 * math.pi * np.outer(i_p, u_n) / 8.0
    W1 = np.where(reim_n[None, :] == 0, np.cos(ang1), np.sin(ang1)) / 8.0
    W1 *= (bi_p[:, None] == bi2_n[None, :])
    W1 = W1.astype(np.float32)

    # C2/S2 [128, 80]: row p=(bj,j); col m=(v, bj2). Horizontal DFT.
    bj_p, j_p = p // 8, p % 8
    m = np.arange(80)
    v_m, bj2_m = m // 16, m % 16
    ang2 = 2.0 * math.pi * np.outer(j_p, v_m) / 8.0
    blk = (bj_p[:, None] == bj2_m[None, :])
    C2 = (np.cos(ang2) * blk).astype(np.float32)
    S2 = (np.sin(ang2) * blk).astype(np.float32)
    z96 = np.zeros((128, 96), dtype=np.float32)
    # padded to N=256 so float32r streams at 1 cycle/row
    CS2P = np.concatenate([C2, S2, z96], axis=1)
    SNC2P = np.concatenate([-S2, C2, z96], axis=1)

    # Wtile [128, 320]: p=(bi,u), f=(v,bj) -> sigmoid(freq_weights)[u*5+v]
    w = 1.0 / (1.0 + np.exp(-freq_weights.astype(np.float64)))
    u_idx = np.arange(128) % 8
    v_idx = np.arange(320) // 64
    Wtile = w[u_idx[:, None] * 5 + v_idx[None, :]].astype(np.float32)
    return W1, CS2P, SNC2P, Wtile


_NC_CACHE = None


def _build_bass(n_imgs: int = IMGS_PER_CORE, repeat: int = 1, cfg: dict = None):
    cfg = dict(cfg or {})
    n_dve_cop = cfg.get("dve_cop", 4)   # chunks 0..n-1 copy on DVE, rest ACT
    n_dve_sq = cfg.get("dve_sq", 0)     # chunks 0..n-1 square on DVE, rest ACT
    add_eng = cfg.get("add", "dve")
    wm_eng = cfg.get("wm", "pool")
    psz_b = cfg.get("psz", 4)
    pso_b = cfg.get("pso", 4)
    ab = cfg.get("a", 8)
    zb = cfg.get("z", 12)
    sqb = cfg.get("sq", 8)
    magb = cfg.get("mag", 8)
    nc = bacc.Bacc("TRN2", target_bir_lowering=False)
    x = nc.dram_tensor("x", [n_imgs * 512, 512], F32R, kind="ExternalInput")
    cst = nc.dram_tensor("cst", [128, 1088], F32R, kind="ExternalInput")
    out = nc.dram_tensor(
        "out", [n_imgs, 64, 64, 64], F32, kind="ExternalOutput"
    )

    # store view: [img, bi_l, s, u, v, bj]; (u,v) merges into one AP dim
    out40 = out[:, 0:40, :, :].rearrange(
        "b (u v) (s p) q -> b p s u v q", u=8, v=5, s=SLABS_PER_IMG, p=16
    )

    with tile.TileContext(nc) as tc, ExitStack() as ctx:
        consts = ctx.enter_context(tc.tile_pool(name="consts", bufs=1))
        a_pool = ctx.enter_context(tc.tile_pool(name="a", bufs=ab))
        z_pool = ctx.enter_context(tc.tile_pool(name="z", bufs=zb))
        sq_pool = ctx.enter_context(tc.tile_pool(name="sq", bufs=sqb))
        mag_pool = ctx.enter_context(tc.tile_pool(name="mag", bufs=magb))
        psz_pool = ctx.enter_context(tc.tile_pool(name="psz", bufs=psz_b, space="PSUM"))
        pso_pool = ctx.enter_context(tc.tile_pool(name="pso", bufs=pso_b, space="PSUM"))

        cst_t = consts.tile([128, 1088], F32R, tag="cst")
        nc.sync.dma_start(cst_t[:], cst[:])
        w1_t = cst_t[:, 0:256]
        cs2_t = cst_t[:, 256:512]
        snc2_t = cst_t[:, 512:768]
        wt_t = cst_t[:, 768:1088]

        def emit_head(img, s):
            a_t = a_pool.tile([128, 512], F32R)
            row0 = img * 512 + s * 128
            nc.scalar.dma_start(a_t[:], x[row0 : row0 + 128, :])
            sq = sq_pool.tile([128, 640], F32, tag="sq")
            # chunks processed in pairs sharing one full 2KB PSUM bank:
            # halves the op count for the PSUM->SBUF copies and squares
            # (per-op fixed overhead ~200ns dominates at these sizes).
            # stage 1 for all chunks first: PE never stalls on the
            # DVE copy of the same chunk's Z
            zps = []
            for pr in range(2):
                psz = psz_pool.tile([128, 512], F32, tag="psz")
                for h in range(2):
                    c = 2 * pr + h
                    nc.tensor.matmul(
                        psz[:, 256 * h : 256 * (h + 1)],
                        a_t[:, 128 * c : 128 * (c + 1)],
                        w1_t,
                        start=True,
                        stop=True,
                    )
                z_p = z_pool.tile([128, 512], F32R)
                nc.vector.tensor_copy(z_p[:], psz[:])
                zps.append(z_p)
            for pr in range(2):
                z_p = zps[pr]
                o2 = pso_pool.tile([128, 512], F32, tag="o2")
                for h in range(2):
                    zre = z_p[:, 256 * h : 256 * h + 128]
                    zim = z_p[:, 256 * h + 128 : 256 * h + 256]
                    dst = o2[:, 256 * h : 256 * (h + 1)]
                    nc.tensor.matmul(dst, zre, cs2_t, start=True, stop=False)
                    nc.tensor.matmul(dst, zim, snc2_t, start=False, stop=True)
                # one ACT square per pair over both chunks' used halves
                nc.scalar.square(
                    sq[:, 320 * pr : 320 * (pr + 1)].rearrange(
                        "p (h g) -> p h g", h=2, g=160
                    ),
                    o2[:].rearrange("p (h w) -> p h w", h=2, w=256)[:, :, 0:160],
                )
            return sq

        def emit_tail(img, s, sq):
            # one add / sqrt / weight-mul per slab (batched over chunks)
            root = mag_pool.tile([128, 320], F32, tag="root")
            ssum = sq_pool.tile([128, 320], F32, tag="ssum")
            sqv = sq[:].rearrange("p (c h g) -> p c h g", c=4, h=2, g=80)
            add_fn = nc.gpsimd.tensor_add if add_eng == "pool" else nc.vector.tensor_add
            add_fn(
                ssum[:].rearrange("p (c g) -> p c g", c=4, g=80),
                sqv[:, :, 0],
                sqv[:, :, 1],
            )
            # write v-major into root: free = v*64 + 16*c + (0..16)
            nc.scalar.sqrt(
                root[:].rearrange("p (v c q) -> p c v q", v=5, c=4, q=16),
                ssum[:].rearrange("p (c v q) -> p c v q", c=4, v=5, q=16),
            )
            magf = mag_pool.tile([128, 320], F32, tag="magf")
            (nc.gpsimd.tensor_mul if wm_eng == "pool" else nc.vector.tensor_mul)(magf[:], root[:], wt_t)
            nc.sync.dma_start(out40[img, :, s], magf[:])

        rep_ctx = tc.For_i(0, repeat, 1) if repeat > 1 else None
        if rep_ctx is not None:
            rep_ctx.__enter__()
        # software-pipelined emission: tail of slab k emitted after head k+1
        depth = cfg.get("depth", 0)
        slabs = [(img, s) for img in range(n_imgs) for s in range(SLABS_PER_IMG)]
        pend = []
        for img, s in slabs:
            sq = emit_head(img, s)
            pend.append((img, s, sq))
            if len(pend) > depth:
                emit_tail(*pend.pop(0))
        while pend:
            emit_tail(*pend.pop(0))
        if rep_ctx is not None:
            rep_ctx.__exit__(None, None, None)
    nc.finalize()
    return nc


def kernel(x: np.ndarray, freq_weights: np.ndarray) -> np.ndarray:
    global _NC_CACHE
    x = np.ascontiguousarray(np.asarray(x, dtype=np.float32))
    freq_weights = np.asarray(freq_weights, dtype=np.float32)
    B = x.shape[0]
    assert x.shape == (32, 1, 512, 512) and freq_weights.shape == (64,)

    W1, CS2P, SNC2P, Wtile = _build_host_matrices(freq_weights)
    cst = np.concatenate([W1, CS2P, SNC2P, Wtile], axis=1)
    if _NC_CACHE is None:
        _NC_CACHE = _build_bass()
    nc = _NC_CACHE

    per = B // N_CORES
    in_maps = []
    for k in range(N_CORES):
        in_maps.append(
            {
                "x": x[k * per : (k + 1) * per].reshape(per * 512, 512),
                "cst": cst,
            }
        )
    res = run_bass_kernel_spmd(nc, in_maps, list(range(N_CORES))).results
    out = np.concatenate([res[k]["out"] for k in range(N_CORES)], axis=0)
    return out.astype(np.float32)


# revision 33
# speedup vs baseline: 1.1507x; 1.0204x over previous
"""Trainium2 Bass kernel for the 8x8-block rfft2 magnitude ("DCT") layer.

Computes, for input x [32,1,512,512] f32 and freq_weights [64] f32:
  per 8x8 spatial block: |rfft2(block, norm='ortho')| -> 40 freq bins,
  scaled by sigmoid(freq_weights)[:40], zero-padded to 64 channels.
Output: [32, 64, 64, 64] f32 (channels 40..63 are zero).

Strategy (pure data parallel, 4 images per core on 8 cores):
  The per-block 2D DFT is separable.  Per 128-row x 512-col slab:
    stage 1 (one matmul per 128-col chunk): data is the *stationary*
      operand, a block-diagonal cos/sin matrix streams:
      Z = A_chunk.T @ W1 -> vertical DFT of every row-block with the
      output transposed so j (intra-block col) is on partitions.
    stage 2 (two accumulating matmuls per chunk): Z_re/Z_im stationary,
      [C2|S2|0] / [-S2|C2|0] streaming -> Re/Im of the 2D DFT laid out
      [(bi,u), (v,bj)], matching output memory order after the (u,v)
      access-pattern dims merge (so one store DMA per slab suffices).
  Matmul operands use float32r (TF32-class, ~2e-4 rel err, 4x rate at
  N>=256).  PSUM->SBUF Z copies on DVE, squares+sqrt on ACT, re/im add
  on DVE, sigmoid-weighting on GPSIMD; input loads on the ACT HWDGE
  ring, one store DMA per slab (256B runs) on the SP HWDGE ring so
  stores never head-of-line block prefetch loads.  Channels 40..63
  rely on the runtime pre-zeroing ExternalOutput buffers.
  Chunks are processed in pairs sharing one full 2KB PSUM bank so the
  PSUM->SBUF copies and ACT squares run as half as many, double-size
  ops (per-op fixed overhead dominates at these sizes).
  Measured ~51us/core steady-state (8 cores x 4 images, concurrent)
  vs ~18-27us memory roofline; engine busy: DMA ~28us, DVE ~27us,
  ACT ~24us, PE ~24us (cost-model sim, matches HW within ~10%).
"""

import math
import numpy as np
from contextlib import ExitStack

import concourse.bacc as bacc
import concourse.mybir as mybir
from concourse import tile
from concourse.bass_utils import run_bass_kernel_spmd

F32 = mybir.dt.float32
F32R = mybir.dt.float32r

N_CORES = 8
IMGS_PER_CORE = 4  # 32 / 8
SLABS_PER_IMG = 4  # 512 rows / 128


def _build_host_matrices(freq_weights: np.ndarray):
    """Block-diagonal DFT coefficient matrices + sigmoid weight tile."""
    p = np.arange(128)
    # W1 [128, 256]: row p=(bi,i); col n=(reim, bi2, u). Vertical DFT, /8.
    bi_p, i_p = p // 8, p % 8
    n = np.arange(256)
    reim_n, r = n // 128, n % 128
    bi2_n, u_n = r // 8, r % 8
    ang1 = 2.0 * math.pi * np.outer(i_p, u_n) / 8.0
    W1 = np.where(reim_n[None, :] == 0, np.cos(ang1), np.sin(ang1)) / 8.0
    W1 *= (bi_p[:, None] == bi2_n[None, :])
    W1 = W1.astype(np.float32)

    # C2/S2 [128, 80]: row p=(bj,j); col m=(v, bj2). Horizontal DFT.
    bj_p, j_p = p // 8, p % 8
    m = np.arange(80)
    v_m, bj2_m = m // 16, m % 16
    ang2 = 2.0 * math.pi * np.outer(j_p, v_m) / 8.0
    blk = (bj_p[:, None] == bj2_m[None, :])
    C2 = (np.cos(ang2) * blk).astype(np.float32)
    S2 = (np.sin(ang2) * blk).astype(np.float32)
    z96 = np.zeros((128, 96), dtype=np.float32)
    # padded to N=256 so float32r streams at 1 cycle/row
    CS2P = np.concatenate([C2, S2, z96], axis=1)
    SNC2P = np.concatenate([-S2, C2, z96], axis=1)

    # Wtile [128, 320]: p=(bi,u), f=(v,bj) -> sigmoid(freq_weights)[u*5+v]
    w = 1.0 / (1.0 + np.exp(-freq_weights.astype(np.float64)))
    u_idx = np.arange(128) % 8
    v_idx = np.arange(320) // 64
    Wtile = w[u_idx[:, None] * 5 + v_idx[None, :]].astype(np.float32)
    return W1, CS2P, SNC2P, Wtile


_NC_CACHE = None


def _build_bass(n_imgs: int = IMGS_PER_CORE, repeat: int = 1, cfg: dict = None):
    cfg = dict(cfg or {})
    n_dve_cop = cfg.get("dve_cop", 4)   # chunks 0..n-1 copy on DVE, rest ACT
    n_dve_sq = cfg.get("dve_sq", 0)     # chunks 0..n-1 square on DVE, rest ACT
    add_eng = cfg.get("add", "dve")
    wm_eng = cfg.get("wm", "pool")
    psz_b = cfg.get("psz", 4)
    pso_b = cfg.get("pso", 4)
    ab = cfg.get("a", 10)
    zb = cfg.get("z", 16)
    sqb = cfg.get("sq", 10)
    magb = cfg.get("mag", 10)
    nc = bacc.Bacc("TRN2", target_bir_lowering=False)
    x = nc.dram_tensor("x", [n_imgs * 512, 512], F32R, kind="ExternalInput")
    cst = nc.dram_tensor("cst", [128, 1088], F32R, kind="ExternalInput")
    out = nc.dram_tensor(
        "out", [n_imgs, 64, 64, 64], F32, kind="ExternalOutput"
    )

    # store view: [img, bi_l, s, u, v, bj]; (u,v) merges into one AP dim
    out40 = out[:, 0:40, :, :].rearrange(
        "b (u v) (s p) q -> b p s u v q", u=8, v=5, s=SLABS_PER_IMG, p=16
    )

    with tile.TileContext(nc) as tc, ExitStack() as ctx:
        consts = ctx.enter_context(tc.tile_pool(name="consts", bufs=1))
        a_pool = ctx.enter_context(tc.tile_pool(name="a", bufs=ab))
        z_pool = ctx.enter_context(tc.tile_pool(name="z", bufs=zb))
        sq_pool = ctx.enter_context(tc.tile_pool(name="sq", bufs=sqb))
        mag_pool = ctx.enter_context(tc.tile_pool(name="mag", bufs=magb))
        psz_pool = ctx.enter_context(tc.tile_pool(name="psz", bufs=psz_b, space="PSUM"))
        pso_pool = ctx.enter_context(tc.tile_pool(name="pso", bufs=pso_b, space="PSUM"))

        cst_t = consts.tile([128, 1088], F32R, tag="cst")
        nc.sync.dma_start(cst_t[:], cst[:])
        w1_t = cst_t[:, 0:256]
        cs2_t = cst_t[:, 256:512]
        snc2_t = cst_t[:, 512:768]
        wt_t = cst_t[:, 768:1088]

        # warm up the ACT function tables (Square, Sqrt) at t=0 so the
        # lazy per-function LoadActFuncSet (~1.3us each) doesn't stall
        # the first slab's magnitude chain mid-ramp
        warm = consts.tile([128, 8], F32, tag="warm")
        nc.gpsimd.memset(warm[:], 0.0)
        nc.scalar.square(warm[:], warm[:])
        nc.scalar.sqrt(warm[:], warm[:])

        def emit_head(img, s):
            a_t = a_pool.tile([128, 512], F32R)
            row0 = img * 512 + s * 128
            nc.scalar.dma_start(a_t[:], x[row0 : row0 + 128, :])
            sq = sq_pool.tile([128, 640], F32, tag="sq")
            # chunks processed in pairs sharing one full 2KB PSUM bank:
            # halves the op count for the PSUM->SBUF copies and squares
            # (per-op fixed overhead ~200ns dominates at these sizes).
            # stage 1 for all chunks first: PE never stalls on the
            # DVE copy of the same chunk's Z
            zps = []
            for pr in range(2):
                psz = psz_pool.tile([128, 512], F32, tag="psz")
                for h in range(2):
                    c = 2 * pr + h
                    nc.tensor.matmul(
                        psz[:, 256 * h : 256 * (h + 1)],
                        a_t[:, 128 * c : 128 * (c + 1)],
                        w1_t,
                        start=True,
                        stop=True,
                    )
                z_p = z_pool.tile([128, 512], F32R)
                nc.vector.tensor_copy(z_p[:], psz[:])
                zps.append(z_p)
            for pr in range(2):
                z_p = zps[pr]
                o2 = pso_pool.tile([128, 512], F32, tag="o2")
                for h in range(2):
                    zre = z_p[:, 256 * h : 256 * h + 128]
                    zim = z_p[:, 256 * h + 128 : 256 * h + 256]
                    dst = o2[:, 256 * h : 256 * (h + 1)]
                    nc.tensor.matmul(dst, zre, cs2_t, start=True, stop=False)
                    nc.tensor.matmul(dst, zim, snc2_t, start=False, stop=True)
                # one ACT square per pair over both chunks' used halves
                nc.scalar.square(
                    sq[:, 320 * pr : 320 * (pr + 1)].rearrange(
                        "p (h g) -> p h g", h=2, g=160
                    ),
                    o2[:].rearrange("p (h w) -> p h w", h=2, w=256)[:, :, 0:160],
                )
            return sq

        def emit_tail(img, s, sq):
            # one add / sqrt / weight-mul per slab (batched over chunks)
            root = mag_pool.tile([128, 320], F32, tag="root")
            ssum = sq_pool.tile([128, 320], F32, tag="ssum")
            sqv = sq[:].rearrange("p (c h g) -> p c h g", c=4, h=2, g=80)
            add_fn = nc.gpsimd.tensor_add if add_eng == "pool" else nc.vector.tensor_add
            add_fn(
                ssum[:].rearrange("p (c g) -> p c g", c=4, g=80),
                sqv[:, :, 0],
                sqv[:, :, 1],
            )
            # write v-major into root: free = v*64 + 16*c + (0..16)
            nc.scalar.sqrt(
                root[:].rearrange("p (v c q) -> p c v q", v=5, c=4, q=16),
                ssum[:].rearrange("p (c v q) -> p c v q", c=4, v=5, q=16),
            )
            magf = mag_pool.tile([128, 320], F32, tag="magf")
            (nc.gpsimd.tensor_mul if wm_eng == "pool" else nc.vector.tensor_mul)(magf[:], root[:], wt_t)
            nc.sync.dma_start(out40[img, :, s], magf[:])

        rep_ctx = tc.For_i(0, repeat, 1) if repeat > 1 else None
        if rep_ctx is not None:
            rep_ctx.__enter__()
        # software-pipelined emission: tail of slab k emitted after head k+1
        depth = cfg.get("depth", 0)
        slabs = [(img, s) for img in range(n_imgs) for s in range(SLABS_PER_IMG)]
        pend = []
        for img, s in slabs:
            sq = emit_head(img, s)
            pend.append((img, s, sq))
            if len(pend) > depth:
                emit_tail(*pend.pop(0))
        while pend:
            emit_tail(*pend.pop(0))
        if rep_ctx is not None:
            rep_ctx.__exit__(None, None, None)
    nc.finalize()
    return nc


def kernel(x: np.ndarray, freq_weights: np.ndarray) -> np.ndarray:
    global _NC_CACHE
    x = np.ascontiguousarray(np.asarray(x, dtype=np.float32))
    freq_weights = np.asarray(freq_weights, dtype=np.float32)
    B = x.shape[0]
    assert x.shape == (32, 1, 512, 512) and freq_weights.shape == (64,)

    W1, CS2P, SNC2P, Wtile = _build_host_matrices(freq_weights)
    cst = np.concatenate([W1, CS2P, SNC2P, Wtile], axis=1)
    if _NC_CACHE is None:
        _NC_CACHE = _build_bass()
    nc = _NC_CACHE

    per = B // N_CORES
    in_maps = []
    for k in range(N_CORES):
        in_maps.append(
            {
                "x": x[k * per : (k + 1) * per].reshape(per * 512, 512),
                "cst": cst,
            }
        )
    res = run_bass_kernel_spmd(nc, in_maps, list(range(N_CORES))).results
    out = np.concatenate([res[k]["out"] for k in range(N_CORES)], axis=0)
    return out.astype(np.float32)


# revision 36
# speedup vs baseline: 1.2162x; 1.0569x over previous
"""Trainium2 Bass kernel for the 8x8-block rfft2 magnitude ("DCT") layer.

Computes, for input x [32,1,512,512] f32 and freq_weights [64] f32:
  per 8x8 spatial block: |rfft2(block, norm='ortho')| -> 40 freq bins,
  scaled by sigmoid(freq_weights)[:40], zero-padded to 64 channels.
Output: [32, 64, 64, 64] f32 (channels 40..63 are zero).

Strategy (pure data parallel, 4 images per core on 8 cores):
  The per-block 2D DFT is separable.  Per 128-row x 512-col slab:
    stage 1 (one matmul per 128-col chunk): data is the *stationary*
      operand, a block-diagonal cos/sin matrix streams:
      Z = A_chunk.T @ W1 -> vertical DFT of every row-block with the
      output transposed so j (intra-block col) is on partitions.
    stage 2 (two accumulating matmuls per chunk): Z_re/Z_im stationary,
      [C2|S2|0] / [-S2|C2|0] streaming -> Re/Im of the 2D DFT laid out
      [(bi,u), (v,bj)], matching output memory order after the (u,v)
      access-pattern dims merge (so one store DMA per slab suffices).
  Matmul operands use float32r (TF32-class, ~2e-4 rel err, 4x rate at
  N>=256).  PSUM->SBUF Z copies on DVE, squares+sqrt on ACT, re/im add
  on DVE, sigmoid-weighting on GPSIMD; input loads on the ACT HWDGE
  ring, one store DMA per slab (256B runs) on the SP HWDGE ring so
  stores never head-of-line block prefetch loads.  Channels 40..63
  rely on the runtime pre-zeroing ExternalOutput buffers.
  Chunks are processed in pairs sharing one full 2KB PSUM bank so the
  PSUM->SBUF copies and ACT squares run as half as many, double-size
  ops (per-op fixed overhead dominates at these sizes).
  ACT function tables (Square/Sqrt) are warmed at t=0 so their lazy
  ~1.3us loads don't stall the first slab's magnitude chain.
  Measured ~50.6us/core steady-state (8 cores x 4 images, concurrent)
  vs ~18-27us memory roofline; engine busy: DMA ~28us, DVE ~27us,
  ACT ~24us, PE ~24us (cost-model sim, matches HW within ~10%).
"""

import math
import numpy as np
from contextlib import ExitStack

import concourse.bacc as bacc
import concourse.mybir as mybir
from concourse import tile
from concourse.bass_utils import run_bass_kernel_spmd

F32 = mybir.dt.float32
F32R = mybir.dt.float32r

N_CORES = 8
IMGS_PER_CORE = 4  # 32 / 8
SLABS_PER_IMG = 4  # 512 rows / 128


def _build_host_matrices(freq_weights: np.ndarray):
    """Block-diagonal DFT coefficient matrices + sigmoid weight tile."""
    p = np.arange(128)
    # W1 [128, 256]: row p=(bi,i); col n=(reim, bi2, u). Vertical DFT, /8.
    bi_p, i_p = p // 8, p % 8
    n = np.arange(256)
    reim_n, r = n // 128, n % 128
    bi2_n, u_n = r // 8, r % 8
    ang1 = 2.0 * math.pi * np.outer(i_p, u_n) / 8.0
    W1 = np.where(reim_n[None, :] == 0, np.cos(ang1), np.sin(ang1)) / 8.0
    W1 *= (bi_p[:, None] == bi2_n[None, :])
    W1 = W1.astype(np.float32)

    # C2/S2 [128, 80]: row p=(bj,j); col m=(v, bj2). Horizontal DFT.
    bj_p, j_p = p // 8, p % 8
    m = np.arange(80)
    v_m, bj2_m = m // 16, m % 16
    ang2 = 2.0 * math.pi * np.outer(j_p, v_m) / 8.0
    blk = (bj_p[:, None] == bj2_m[None, :])
    C2 = (np.cos(ang2) * blk).astype(np.float32)
    S2 = (np.sin(ang2) * blk).astype(np.float32)
    z96 = np.zeros((128, 96), dtype=np.float32)
    # padded to N=256 so float32r streams at 1 cycle/row
    CS2P = np.concatenate([C2, S2, z96], axis=1)
    SNC2P = np.concatenate([-S2, C2, z96], axis=1)

    # Wtile [128, 320]: p=(bi,u), f=(v,bj) -> sigmoid(freq_weights)[u*5+v]
    w = 1.0 / (1.0 + np.exp(-freq_weights.astype(np.float64)))
    u_idx = np.arange(128) % 8
    v_idx = np.arange(320) // 64
    Wtile = w[u_idx[:, None] * 5 + v_idx[None, :]].astype(np.float32)
    return W1, CS2P, SNC2P, Wtile


_NC_CACHE = None


def _build_bass(n_imgs: int = IMGS_PER_CORE, repeat: int = 1, cfg: dict = None):
    cfg = dict(cfg or {})
    n_dve_cop = cfg.get("dve_cop", 4)   # chunks 0..n-1 copy on DVE, rest ACT
    n_dve_sq = cfg.get("dve_sq", 0)     # chunks 0..n-1 square on DVE, rest ACT
    add_eng = cfg.get("add", "dve")
    wm_eng = cfg.get("wm", "pool")
    psz_b = cfg.get("psz", 4)
    pso_b = cfg.get("pso", 4)
    ab = cfg.get("a", 10)
    zb = cfg.get("z", 16)
    sqb = cfg.get("sq", 10)
    magb = cfg.get("mag", 10)
    nc = bacc.Bacc("TRN2", target_bir_lowering=False)
    x = nc.dram_tensor("x", [n_imgs * 512, 512], F32R, kind="ExternalInput")
    cst = nc.dram_tensor("cst", [128, 1088], F32R, kind="ExternalInput")
    out = nc.dram_tensor(
        "out", [n_imgs, 64, 64, 64], F32, kind="ExternalOutput"
    )

    # store view: [img, bi_l, s, u, v, bj]; (u,v) merges into one AP dim
    out40 = out[:, 0:40, :, :].rearrange(
        "b (u v) (s p) q -> b p s u v q", u=8, v=5, s=SLABS_PER_IMG, p=16
    )

    with tile.TileContext(nc) as tc, ExitStack() as ctx:
        consts = ctx.enter_context(tc.tile_pool(name="consts", bufs=1))
        a_pool = ctx.enter_context(tc.tile_pool(name="a", bufs=ab))
        z_pool = ctx.enter_context(tc.tile_pool(name="z", bufs=zb))
        sq_pool = ctx.enter_context(tc.tile_pool(name="sq", bufs=sqb))
        mag_pool = ctx.enter_context(tc.tile_pool(name="mag", bufs=magb))
        psz_pool = ctx.enter_context(tc.tile_pool(name="psz", bufs=psz_b, space="PSUM"))
        pso_pool = ctx.enter_context(tc.tile_pool(name="pso", bufs=pso_b, space="PSUM"))

        cst_t = consts.tile([128, 1088], F32R, tag="cst")
        # w1 first: the first stage-1 matmul only needs columns 0:256,
        # so don't gate it on the full 557KB constant transfer
        nc.sync.dma_start(cst_t[:, 0:256], cst[:, 0:256])
        nc.sync.dma_start(cst_t[:, 256:1088], cst[:, 256:1088])
        w1_t = cst_t[:, 0:256]
        cs2_t = cst_t[:, 256:512]
        snc2_t = cst_t[:, 512:768]
        wt_t = cst_t[:, 768:1088]

        # prefetch the first two slabs' input loads before the ACT warmup
        # ops so the warmup table loads don't block them on the ACT queue
        slabs = [(img, s) for img in range(n_imgs) for s in range(SLABS_PER_IMG)]
        pre_a = {}
        for img, s in slabs[:2]:
            a_t = a_pool.tile([128, 512], F32R)
            row0 = img * 512 + s * 128
            nc.scalar.dma_start(a_t[:], x[row0 : row0 + 128, :])
            pre_a[(img, s)] = a_t

        # warm up the ACT function tables (Square, Sqrt) at t=0 so the
        # lazy per-function LoadActFuncSet (~1.3us each) doesn't stall
        # the first slab's magnitude chain mid-ramp
        warm = consts.tile([128, 8], F32, tag="warm")
        nc.gpsimd.memset(warm[:], 0.0)
        nc.scalar.square(warm[:], warm[:])
        nc.scalar.sqrt(warm[:], warm[:])

        def emit_head(img, s, a_t=None):
            if a_t is None:
                a_t = a_pool.tile([128, 512], F32R)
                row0 = img * 512 + s * 128
                nc.scalar.dma_start(a_t[:], x[row0 : row0 + 128, :])
            sq = sq_pool.tile([128, 640], F32, tag="sq")
            # chunks processed in pairs sharing one full 2KB PSUM bank:
            # halves the op count for the PSUM->SBUF copies and squares
            # (per-op fixed overhead ~200ns dominates at these sizes).
            # stage 1 for all chunks first: PE never stalls on the
            # DVE copy of the same chunk's Z
            zps = []
            for pr in range(2):
                psz = psz_pool.tile([128, 512], F32, tag="psz")
                for h in range(2):
                    c = 2 * pr + h
                    nc.tensor.matmul(
                        psz[:, 256 * h : 256 * (h + 1)],
                        a_t[:, 128 * c : 128 * (c + 1)],
                        w1_t,
                        start=True,
                        stop=True,
                    )
                z_p = z_pool.tile([128, 512], F32R)
                nc.vector.tensor_copy(z_p[:], psz[:])
                zps.append(z_p)
            for pr in range(2):
                z_p = zps[pr]
                o2 = pso_pool.tile([128, 512], F32, tag="o2")
                for h in range(2):
                    zre = z_p[:, 256 * h : 256 * h + 128]
                    zim = z_p[:, 256 * h + 128 : 256 * h + 256]
                    dst = o2[:, 256 * h : 256 * (h + 1)]
                    nc.tensor.matmul(dst, zre, cs2_t, start=True, stop=False)
                    nc.tensor.matmul(dst, zim, snc2_t, start=False, stop=True)
                # one ACT square per pair over both chunks' used halves
                nc.scalar.square(
                    sq[:, 320 * pr : 320 * (pr + 1)].rearrange(
                        "p (h g) -> p h g", h=2, g=160
                    ),
                    o2[:].rearrange("p (h w) -> p h w", h=2, w=256)[:, :, 0:160],
                )
            return sq

        def emit_tail(img, s, sq):
            # one add / sqrt / weight-mul per slab (batched over chunks)
            root = mag_pool.tile([128, 320], F32, tag="root")
            ssum = sq_pool.tile([128, 320], F32, tag="ssum")
            sqv = sq[:].rearrange("p (c h g) -> p c h g", c=4, h=2, g=80)
            add_fn = nc.gpsimd.tensor_add if add_eng == "pool" else nc.vector.tensor_add
            add_fn(
                ssum[:].rearrange("p (c g) -> p c g", c=4, g=80),
                sqv[:, :, 0],
                sqv[:, :, 1],
            )
            # write v-major into root: free = v*64 + 16*c + (0..16)
            nc.scalar.sqrt(
                root[:].rearrange("p (v c q) -> p c v q", v=5, c=4, q=16),
                ssum[:].rearrange("p (c v q) -> p c v q", c=4, v=5, q=16),
            )
            magf = mag_pool.tile([128, 320], F32, tag="magf")
            (nc.gpsimd.tensor_mul if wm_eng == "pool" else nc.vector.tensor_mul)(magf[:], root[:], wt_t)
            nc.sync.dma_start(out40[img, :, s], magf[:])

        rep_ctx = tc.For_i(0, repeat, 1) if repeat > 1 else None
        if rep_ctx is not None:
            rep_ctx.__enter__()
        # software-pipelined emission: tail of slab k emitted after head k+1
        depth = cfg.get("depth", 0)
        pend = []
        for img, s in slabs:
            sq = emit_head(img, s, pre_a.pop((img, s), None))
            pend.append((img, s, sq))
            if len(pend) > depth:
                emit_tail(*pend.pop(0))
        while pend:
            emit_tail(*pend.pop(0))
        if rep_ctx is not None:
            rep_ctx.__exit__(None, None, None)
    nc.finalize()
    return nc


def kernel(x: np.ndarray, freq_weights: np.ndarray) -> np.ndarray:
    global _NC_CACHE
    x = np.ascontiguousarray(np.asarray(x, dtype=np.float32))
    freq_weights = np.asarray(freq_weights, dtype=np.float32)
    B = x.shape[0]
    assert x.shape == (32, 1, 512, 512) and freq_weights.shape == (64,)

    W1, CS2P, SNC2P, Wtile = _build_host_matrices(freq_weights)
    cst = np.concatenate([W1, CS2P, SNC2P, Wtile], axis=1)
    if _NC_CACHE is None:
        _NC_CACHE = _build_bass()
    nc = _NC_CACHE

    per = B // N_CORES
    in_maps = []
    for k in range(N_CORES):
        in_maps.append(
            {
                "x": x[k * per : (k + 1) * per].reshape(per * 512, 512),
                "cst": cst,
            }
        )
    res = run_bass_kernel_spmd(nc, in_maps, list(range(N_CORES))).results
    out = np.concatenate([res[k]["out"] for k in range(N_CORES)], axis=0)
    return out.astype(np.float32)
